# revision 40
# baseline (speedup 1.0000x reference)
"""Trainium2 Bass kernel for nn_Agent_Aggregator_with_Mask_Denoise_Mechanism.

Sharding: tensor-parallel over heads h (8 heads -> 8 cores). Each core computes
its head end-to-end; the only cross-core value is the scalar threshold logit
(an 8-way AllReduce of a 4-byte partial sum, padded to 64 B). Host does the
final (h d) concat + transpose.

Math notes vs the reference:
- sigmoid(m) > sigmoid(t)  <=>  m > t, so the mask threshold compares
  pre-sigmoid logits and no sigmoid tables are needed (Exp only).
- The reference's kv.reshape(b, a, h*d) row-major remap means the thresh
  weight applied to kv[b,h,a,d] is w_thresh[0, (a%8)*64+d], independent of h.
- Softmax normalizations are folded into matmuls (ones columns / ones blocks);
  each division happens on the small side of its matmul.
- ka softmax skips max-subtraction (fp32 psum); a fixed shift of -14 keeps
  exp outputs inside fp16 range for the kv matmul.

Schedule notes:
- The AllReduce costs ~40us wall regardless of payload, so it is launched
  right after the kv epilogue (phase D) and overlapped with the qa phase
  (F: logits+exp) plus the qa-softmax denominators (H-s) and the
  threshold-independent parts of the mask/denoise phase (G-pre).
- v projections for two adjacent 512-col chunks run as a PE column-group
  pair (tile_position (0,0)/(0,64)) so both halves of the PE array stream
  concurrently; same pattern as the paired output matmuls in phase H.
"""
import os
import sys

sys.path.insert(0, "/opt/trn_rl_repo")

import numpy as np
from contextlib import ExitStack

import concourse.bass as bass
import concourse.tile as tile
from concourse import bacc, mybir, bass_utils

f32 = mybir.dt.float32
f16 = mybir.dt.float16

B, N, DIM = 2, 8192, 512
H, A, D = 8, 256, 64
N_CORES = 8
NBLK = 4            # 2048-column blocks per batch
SHIFT = -14.0       # ka exp shift to fit fp16

_cache = {}


def _install_profile_shim():
    """Restore the axon NTFF profile hook + disable artifact upload."""
    import contextlib
    import ctypes
    import types

    if "antenv.axon_hooks" in sys.modules:
        return
    so_path = "/opt/axon/libaxon_pjrt.so"
    holder = [None]
    mod = types.ModuleType("antenv.axon_hooks")
    mod.set_axon_ntff_profile_hook = lambda h: holder.__setitem__(0, h)
    mod.get_axon_ntff_profile_hook = lambda: holder[0]
    sys.modules["antenv.axon_hooks"] = mod
    try:
        lib = ctypes.CDLL(so_path)
        if hasattr(lib, "axon_start_nrt_profile"):
            lib.axon_start_nrt_profile.argtypes = [
                ctypes.POINTER(ctypes.c_int64),
                ctypes.c_size_t,
            ]
            lib.axon_start_nrt_profile.restype = ctypes.c_int64
            lib.axon_stop_nrt_profile.argtypes = [ctypes.c_char_p]
            lib.axon_stop_nrt_profile.restype = ctypes.c_int64

            @contextlib.contextmanager
            def _hook(output_dir, device_ids):
                import jax

                jax.devices()
                if device_ids:
                    ids = (ctypes.c_int64 * len(device_ids))(*device_ids)
                    rc = lib.axon_start_nrt_profile(ids, len(device_ids))
                else:
                    rc = lib.axon_start_nrt_profile(None, 0)
                if rc != 0:
                    raise RuntimeError(f"axon_start_nrt_profile rc={rc}")
                try:
                    yield
                finally:
                    n = lib.axon_stop_nrt_profile(str(output_dir).encode())
                    if n < 0:
                        raise RuntimeError(f"axon_stop_nrt_profile rc={n}")

            mod.set_axon_ntff_profile_hook(_hook)
    except OSError:
        pass
    bass_utils.upload_artifacts = lambda tmpdir: f"file://{tmpdir}"


def _build():
    nc = bacc.Bacc("TRN2", target_bir_lowering=False, debug=False,
                   num_devices=N_CORES)

    XT = nc.dram_tensor("xt", [B * 4, 128, N], f16, kind="ExternalInput").ap()
    WQK = nc.dram_tensor("wqk", [4, 128, 128], f16, kind="ExternalInput").ap()
    WV = nc.dram_tensor("wv", [4, 128, 64], f16, kind="ExternalInput").ap()
    AGS = nc.dram_tensor("ags", [128, 256], f16, kind="ExternalInput").ap()
    WN = nc.dram_tensor("wn", [64, 64], f32, kind="ExternalInput").ap()
    WM = nc.dram_tensor("wm", [64, 64], f32, kind="ExternalInput").ap()
    BN = nc.dram_tensor("bn", [128, 64], f32, kind="ExternalInput").ap()
    BM = nc.dram_tensor("bm", [128, 64], f32, kind="ExternalInput").ap()
    WTT = nc.dram_tensor("wtt", [2, 128, 64], f32, kind="ExternalInput").ap()
    IDENT = nc.dram_tensor("ident", [128, 128], f32, kind="ExternalInput").ap()
    BTHR = nc.dram_tensor("bthr", [128, 1], f32, kind="ExternalInput").ap()
    VONES = nc.dram_tensor("vones", [16, N], f16, kind="ExternalInput").ap()
    OUT = nc.dram_tensor("out_t", [B, 8, 128, 512], f16,
                         kind="ExternalOutput").ap()

    EXP = mybir.ActivationFunctionType.Exp
    MULT = mybir.AluOpType.mult
    ADD = mybir.AluOpType.add
    GT = mybir.AluOpType.is_gt

    with tile.TileContext(nc) as tc, ExitStack() as ctx:
        const = ctx.enter_context(tc.tile_pool(name="const", bufs=1))
        big = ctx.enter_context(tc.tile_pool(name="big", bufs=1))
        ek = ctx.enter_context(tc.tile_pool(name="ek", bufs=3))
        dsb = ctx.enter_context(tc.tile_pool(name="dsb", bufs=1))
        hsb = ctx.enter_context(tc.tile_pool(name="hsb", bufs=2))
        dram = ctx.enter_context(tc.tile_pool(name="dram", bufs=1, space="DRAM"))
        ps_qk = ctx.enter_context(tc.tile_pool(name="ps_qk", bufs=2, space="PSUM"))
        ps_v = ctx.enter_context(tc.tile_pool(name="ps_v", bufs=2, space="PSUM"))
        ps_lg = ctx.enter_context(tc.tile_pool(name="ps_lg", bufs=2, space="PSUM"))
        ps_kvt = ctx.enter_context(tc.tile_pool(name="ps_kvt", bufs=2, space="PSUM"))

        # ---- constants to SBUF
        wqk_sb = []
        wv_sb = []
        for dc in range(4):
            w1 = const.tile([128, 128], f16, name=f"wqk{dc}")
            nc.scalar.dma_start(w1[:], WQK[dc])
            wqk_sb.append(w1)
            w2 = const.tile([128, 64], f16, name=f"wv{dc}")
            nc.scalar.dma_start(w2[:], WV[dc])
            wv_sb.append(w2)
        ags_sb = const.tile([128, 256], f16)
        nc.scalar.dma_start(ags_sb[:], AGS[:])
        wn_sb = const.tile([64, 64], f32)
        nc.scalar.dma_start(wn_sb[:], WN[:])
        wm_sb = const.tile([64, 64], f32)
        nc.scalar.dma_start(wm_sb[:], WM[:])
        bn_sb = const.tile([128, 64], f32)
        nc.scalar.dma_start(bn_sb[:], BN[:])
        bm_sb = const.tile([128, 64], f32)
        nc.scalar.dma_start(bm_sb[:], BM[:])
        wtt_sb = []
        for ac in range(2):
            w3 = const.tile([128, 64], f32, name=f"wtt{ac}")
            nc.scalar.dma_start(w3[:], WTT[ac])
            wtt_sb.append(w3)
        id_sb = const.tile([128, 128], f32)
        nc.scalar.dma_start(id_sb[:], IDENT[:])
        bthr_sb = const.tile([128, 1], f32)
        nc.scalar.dma_start(bthr_sb[:], BTHR[:])
        bias_sh = const.tile([128, 1], f32)
        nc.vector.memset(bias_sh[:], SHIFT)
        ones64 = const.tile([128, 64], f16)
        nc.vector.memset(ones64[:], 1.0)
        ones128 = nc.const_aps.tensor(1.0, [128, 1])

        # warmup collective input (the collective itself is emitted after the
        # x prefetch loop so it does not block the gpsimd DMA queue early)
        warm_sb = const.tile([1, 16], f32)
        nc.vector.memset(warm_sb[:], 0.0)
        warm_in = dram.tile([1, 16], f32)
        warm_out = dram.tile([1, 16], f32, addr_space="Shared")
        nc.sync.dma_start(warm_in[:], warm_sb[:])

        # ---- persistent big tiles
        qkT = [big.tile([128, N], f16, name=f"qkT{b}") for b in range(B)]
        vsb = [big.tile([128, 64 * 80], f16, name=f"vsb{b}") for b in range(B)]
        vsb3 = [t[:].rearrange("p (c e) -> p c e", e=80) for t in vsb]
        eqa = [big.tile([128, N], f16, name=f"eqa{b}{ac}")
               for b in range(B) for ac in range(2)]
        rso_sb = [big.tile([128, 512], f16, name=f"rso{b}{pr}")
                  for b in range(B) for pr in range(8)]

        # ---- phase-D state (filled by emit_d)
        noise_sb = {}
        mask_sb = {}
        kv_sb = {}
        rs_sb = {}
        r_sb = {}
        zm_sb = {}
        den_sb = {}

        def emit_d(b, kvt_ps):
            t_u = dsb.tile([65, 256], f32, name=f"kvut{b}")
            nc.vector.tensor_copy(t_u[:], kvt_ps[b][:])
            for ac in range(2):
                asl = slice(ac * 128, (ac + 1) * 128)
                sm1 = ps_v.tile([128, 65], f32, name="sm1", tag="vps")
                nc.tensor.matmul(sm1[:, 0:64], t_u[0:64, asl], wn_sb[:],
                                 start=True, stop=True)
                t_n = dsb.tile([128, 64], f32, name=f"noise{b}{ac}")
                nc.vector.tensor_copy(t_n[:], sm1[:, 0:64])
                noise_sb[b, ac] = t_n
                sm2 = ps_v.tile([128, 65], f32, name="sm2", tag="vps")
                nc.tensor.matmul(sm2[:, 0:64], t_u[0:64, asl], wm_sb[:],
                                 start=True, stop=True)
                t_m = dsb.tile([128, 64], f32, name=f"mask{b}{ac}")
                nc.vector.tensor_copy(t_m[:], sm2[:, 0:64])
                mask_sb[b, ac] = t_m
                sm3 = ps_v.tile([128, 65], f32, name="sm3", tag="vps")
                nc.tensor.transpose(sm3[:], t_u[:, asl], id_sb[0:65, 0:65])
                t_k = dsb.tile([128, 65], f32, name=f"kvn{b}{ac}")
                nc.vector.tensor_copy(t_k[:], sm3[:])
                t_rs = dsb.tile([128, 1], f32, name=f"rs{b}{ac}")
                nc.vector.reciprocal_approx_fast(t_rs[:], t_k[:, 64:65])
                rs_sb[b, ac] = t_rs
                t_kv = dsb.tile([128, 64], f32, name=f"kv{b}{ac}")
                nc.vector.tensor_scalar(out=t_kv[:], in0=t_k[:, 0:64],
                                        scalar1=t_rs[:], scalar2=None, op0=MULT)
                kv_sb[b, ac] = t_kv
                t_tmp = dsb.tile([128, 64], f32, name=f"tt{b}{ac}")
                nc.vector.tensor_tensor(t_tmp[:], t_kv[:], wtt_sb[ac][:], MULT)
                t_r = dsb.tile([128, 1], f32, name=f"r{b}{ac}")
                nc.vector.tensor_reduce(t_r[:], t_tmp[:],
                                        axis=mybir.AxisListType.X, op=ADD)
                r_sb[b, ac] = t_r

        def emit_g_pre(b, ac):
            # threshold-independent parts of the mask/denoise epilogue
            t_rs = rs_sb[b, ac]
            zm = dsb.tile([128, 64], f32, name=f"zm{b}{ac}")
            nc.vector.scalar_tensor_tensor(
                out=zm[:], in0=mask_sb[b, ac][:], scalar=t_rs[:],
                in1=bm_sb[:], op0=MULT, op1=ADD)
            zm_sb[b, ac] = zm
            gn = dsb.tile([128, 64], f32, name=f"gn{b}{ac}")
            nc.vector.scalar_tensor_tensor(
                out=gn[:], in0=noise_sb[b, ac][:], scalar=t_rs[:],
                in1=bn_sb[:], op0=MULT, op1=ADD)
            en = dsb.tile([128, 64], f32, name=f"en{b}{ac}")
            nc.scalar.activation(en[:], gn[:], EXP, scale=-1.0)
            dd = dsb.tile([128, 64], f32, name=f"dd{b}{ac}")
            nc.vector.tensor_scalar(out=dd[:], in0=en[:], scalar1=1.0,
                                    scalar2=None, op0=ADD)
            den = dsb.tile([128, 64], f32, name=f"den{b}{ac}")
            nc.vector.reciprocal_approx_fast(den[:], dd[:])
            den_sb[b, ac] = den

        def emit_f(b, pr):
            # qa logits + exp for one 1024-token pair, then the qa-softmax
            # denominator matmuls (H-s) and their reciprocal.
            sl0 = slice((2 * pr) * 512, (2 * pr + 1) * 512)
            sl1 = slice((2 * pr + 1) * 512, (2 * pr + 2) * 512)
            for sl in (sl0, sl1):
                for ac in range(2):
                    lgq = ps_lg.tile([128, 512], f32, name="lgq", tag="lg")
                    nc.tensor.matmul(
                        lgq[:], ags_sb[0:64, ac * 128:(ac + 1) * 128],
                        qkT[b][0:64, sl], start=True, stop=True)
                    nc.scalar.activation(eqa[b * 2 + ac][:, sl], lgq[:], EXP)
            s_ps = ps_qk.tile([128, 512], f32, name="sps", tag="qkps")
            for ac in range(2):
                eq = eqa[b * 2 + ac]
                nc.tensor.matmul(s_ps[0:64, :], ones64[:], eq[:, sl0],
                                 start=(ac == 0), stop=(ac == 1),
                                 tile_position=(0, 0))
                nc.tensor.matmul(s_ps[64:128, :], ones64[:], eq[:, sl1],
                                 start=(ac == 0), stop=(ac == 1),
                                 tile_position=(0, 64))
            rtmp = hsb.tile([128, 512], f32, name="rtmp", tag="rtmp", bufs=1)
            nc.vector.reciprocal_approx_fast(rtmp[:], s_ps[:])
            nc.vector.tensor_copy(rso_sb[b * 8 + pr][:], rtmp[:])

        # ====== main loop: A (proj) + B (v transpose) + C (ka) ======
        N_F_EARLY = 3   # F groups for b=0 emitted inside the last b=1 block
        with ExitStack() as sA:
            xtp = sA.enter_context(tc.tile_pool(name="xtp", bufs=2))
            kvt_ps = [ps_kvt.tile([65, 256], f32, name=f"kvtps{b}", tag="kvtps")
                      for b in range(B)]
            kv_mm_idx = [0, 0]
            # pre-issue all x loads on gpsimd (SWDGE) so they never queue
            # behind exp/transpose work; pool WAR deps pace the prefetch.
            # x prefetch: half-block [128, 1024] tiles.  dc2/dc3 go on the
            # sync queue (paced by pool WARs); dc0/dc1 go on the scalar
            # queue, emitted 3 iterations ahead inside the main loop so
            # their WAR deps are always satisfied and never block the
            # queue.  gpsimd carries no loads -- it is dedicated to the
            # collectives.
            xts_all = {}
            pairs = [(blk, b) for blk in range(NBLK) for b in range(B)]

            def load_x01(it):
                blk, b = pairs[it]
                for hf in range(2):
                    bsl = slice(blk * 2048 + hf * 1024,
                                blk * 2048 + (hf + 1) * 1024)
                    for dc in range(2):
                        xt_t = xtp.tile([128, 1024], f16,
                                        name=f"x{blk}{b}{dc}{hf}",
                                        tag=f"x{dc}", bufs=6)
                        nc.scalar.dma_start(xt_t[:], XT[b * 4 + dc][:, bsl])
                        xts_all[blk, b, dc, hf] = xt_t

            for blk in range(NBLK):
                for b in range(B):
                    for hf in range(2):
                        bsl = slice(blk * 2048 + hf * 1024,
                                    blk * 2048 + (hf + 1) * 1024)
                        for dc in range(2, 4):
                            xt_t = xtp.tile([128, 1024], f16,
                                            name=f"x{blk}{b}{dc}{hf}",
                                            tag=f"x{dc}", bufs=3)
                            nc.sync.dma_start(xt_t[:], XT[b * 4 + dc][:, bsl])
                            xts_all[blk, b, dc, hf] = xt_t
            for it in range(3):
                load_x01(it)

            # warmup collective on the (otherwise idle) gpsimd queue: it
            # pre-initializes the CC stream behind the framework init
            # barrier, so the real AllReduce below starts with ~1us latency
            # instead of ~12us and runs ~2x faster.  Result is unused.
            nc.gpsimd.collective_compute(
                "AllReduce", ADD, ins=[warm_in[:]], outs=[warm_out[:]],
                replica_groups=[list(range(N_CORES))])

            # persistent ping-pong vt tiles; their VONES rows are loaded once
            vts = [big.tile([80, 2048], f16, name=f"vt{i}") for i in range(2)]
            for i in range(2):
                nc.sync.dma_start(vts[i][64:80, :], VONES[:, 0:2048])

            def emit_v(it):
                # v projection + transposes for iteration `it`, software-
                # pipelined one iteration ahead of its consuming C phase.
                blk, b = pairs[it]
                vt = vts[it % 2]
                for pr2 in range(2):
                    sse = slice((2 * pr2) * 512, (2 * pr2 + 1) * 512)
                    sso = slice((2 * pr2 + 1) * 512, (2 * pr2 + 2) * 512)
                    xh = [xts_all[blk, b, dc, pr2] for dc in range(4)]
                    v2_ps = ps_v.tile([128, 512], f32, name="vps", tag="vps")
                    for dc in range(4):
                        nc.tensor.matmul(v2_ps[0:64, :], wv_sb[dc][:],
                                         xh[dc][:, 0:512],
                                         start=(dc == 0), stop=(dc == 3),
                                         tile_position=(0, 0))
                        nc.tensor.matmul(v2_ps[64:128, :], wv_sb[dc][:],
                                         xh[dc][:, 512:1024],
                                         start=(dc == 0), stop=(dc == 3),
                                         tile_position=(0, 64))
                    nc.vector.tensor_copy(vt[0:64, sse], v2_ps[0:64, :])
                    nc.vector.tensor_copy(vt[0:64, sso], v2_ps[64:128, :])
                    hsl = slice(pr2 * 1024, (pr2 + 1) * 1024)
                    nc.sync.dma_start_transpose(
                        vsb3[b][:, blk * 16 + pr2 * 8:
                                 blk * 16 + (pr2 + 1) * 8, :], vt[:, hsl])

            for it, (blk, b) in enumerate(pairs):
                emit_v(it)
                if it + 3 < len(pairs):
                    load_x01(it + 3)
                for sc in range(4):
                    nck = blk * 4 + sc
                    sl = slice(nck * 512, (nck + 1) * 512)
                    qk_ps = ps_qk.tile([128, 512], f32, name="qkps",
                                       tag="qkps")
                    for dc in range(4):
                        nc.tensor.matmul(
                            qk_ps[:], wqk_sb[dc][:],
                            xts_all[blk, b, dc, sc // 2][
                                :, (sc % 2) * 512:(sc % 2 + 1) * 512],
                            start=(dc == 0), stop=(dc == 3))
                    nc.vector.tensor_copy(qkT[b][:, sl], qk_ps[:])
                # early F groups for b=0 fill the end-of-loop drain while
                # the last block's transpose + C chain completes
                if it == len(pairs) - 1:
                    for pr in range(N_F_EARLY):
                        emit_f(0, pr)
                # C: ka logits -> exp -> kv^T
                for cp in range(blk * 8, (blk + 1) * 8):
                    lg = ps_lg.tile([128, 512], f32, name="lg", tag="lg")
                    for j in range(2):
                        c = 2 * cp + j
                        nc.tensor.matmul(
                            lg[:, j * 256:(j + 1) * 256],
                            qkT[b][64:128, c * 128:(c + 1) * 128],
                            ags_sb[64:128, :],
                            start=True, stop=True)
                    e_t = ek.tile([128, 512], f16, name="eka", tag="eka")
                    nc.scalar.activation(e_t[:], lg[:], EXP, bias=bias_sh[:])
                    for j in range(2):
                        c = 2 * cp + j
                        ki = kv_mm_idx[b]
                        nc.tensor.matmul(
                            kvt_ps[b][:], vsb3[b][:, c, 0:65],
                            e_t[:, j * 256:(j + 1) * 256],
                            start=(ki == 0), stop=(ki == 63))
                        kv_mm_idx[b] += 1
                # D: per-batch epilogue right after its last C block
                if blk == NBLK - 1:
                    emit_d(b, kvt_ps)

            # ---- threshold partial + cross-core AllReduce (launch ASAP;
            # phases F / H-s / G-pre below overlap its ~40us latency)
            th_ps = ps_v.tile([1, 16], f32, name="thps", tag="vps")
            k = 0
            for b in range(B):
                for ac in range(2):
                    nc.tensor.matmul(th_ps[0:1, 0:1], r_sb[b, ac][:],
                                     ones128[0:128, :],
                                     start=(k == 0), stop=(k == 3))
                    k += 1
            th_sb = dsb.tile([1, 16], f32)
            nc.vector.memset(th_sb[:], 0.0)
            nc.vector.tensor_copy(th_sb[0:1, 0:1], th_ps[0:1, 0:1])
            cc_in = dram.tile([1, 16], f32)
            cc_out = dram.tile([1, 16], f32, addr_space="Shared")
            nc.sync.dma_start(cc_in[:], th_sb[:])
            nc.gpsimd.collective_compute(
                "AllReduce", ADD, ins=[cc_in[:]], outs=[cc_out[:]],
                replica_groups=[list(range(N_CORES))])

            # ---- G-pre: threshold-independent mask/denoise pieces
            for b in range(B):
                for ac in range(2):
                    emit_g_pre(b, ac)

            # ---- F: qa logits -> exp, plus qa-softmax denominators (H-s),
            # all overlapped with the AllReduce.
            for b in range(B):
                for pr in range(N_F_EARLY if b == 0 else 0, 8):
                    emit_f(b, pr)

            # ---- collective result -> threshold scalar
            ts_sb = dsb.tile([1, 16], f32)
            nc.sync.dma_start(ts_sb[:], cc_out[:])
            tbc = dsb.tile([128, 1], f32)
            nc.gpsimd.partition_broadcast(tbc[:], ts_sb[0:1, 0:1])
            tfin = dsb.tile([128, 1], f32)
            nc.vector.tensor_scalar(out=tfin[:], in0=tbc[:],
                                    scalar1=1.0 / (B * A), scalar2=bthr_sb[:],
                                    op0=MULT, op1=ADD)

        # ========== phase G: mask -> kv2 (only thresh-dependent part),
        # interleaved per batch with phase H so H(b=0) starts while G(b=1)
        # still runs on the vector/scalar engines.
        kv2_sb = {}

        def emit_g(b):
            for ac in range(2):
                t_kv = kv_sb[b, ac]
                mb = dsb.tile([128, 64], f32, name=f"mb{b}{ac}")
                nc.vector.tensor_scalar(out=mb[:], in0=zm_sb[b, ac][:],
                                        scalar1=tfin[:],
                                        scalar2=None, op0=GT)
                kvm = dsb.tile([128, 64], f32, name=f"kvm{b}{ac}")
                nc.vector.tensor_tensor(kvm[:], t_kv[:], mb[:], MULT)
                l2 = dsb.tile([128, 64], f32, name=f"l2{b}{ac}")
                nc.vector.tensor_tensor(l2[:], kvm[:], den_sb[b, ac][:], ADD)
                e2 = dsb.tile([128, 64], f32, name=f"e2{b}{ac}")
                s2 = dsb.tile([128, 1], f32, name=f"s2{b}{ac}")
                nc.scalar.activation(e2[:], l2[:], EXP, accum_out=s2[:])
                rs2 = dsb.tile([128, 1], f32, name=f"rs2{b}{ac}")
                nc.vector.reciprocal_approx_fast(rs2[:], s2[:])
                kv2 = dsb.tile([128, 64], f16, name=f"kv2_{b}{ac}")
                nc.vector.tensor_scalar(out=kv2[:], in0=e2[:],
                                        scalar1=rs2[:], scalar2=None, op0=MULT)
                kv2_sb[b * 2 + ac] = kv2

        # ===== phase H: out^T = kv2^T @ E_qa^T, paired via column tiling ====
        # Chunk pairs land on disjoint PSUM partition halves.  Denominators
        # were already folded into rso_sb during phase F.
        emit_g(0)
        for b in range(B):
            if b + 1 < B:
                emit_g(b + 1)
            for pr in range(8):
                sl0 = slice((2 * pr) * 512, (2 * pr + 1) * 512)
                sl1 = slice((2 * pr + 1) * 512, (2 * pr + 2) * 512)
                o_ps = ps_lg.tile([128, 512], f32, name="ops", tag="lg")
                for ac in range(2):
                    k2 = kv2_sb[b * 2 + ac]
                    eq = eqa[b * 2 + ac]
                    nc.tensor.matmul(o_ps[0:64, :], k2[:], eq[:, sl0],
                                     start=(ac == 0), stop=(ac == 1),
                                     tile_position=(0, 0))
                    nc.tensor.matmul(o_ps[64:128, :], k2[:], eq[:, sl1],
                                     start=(ac == 0), stop=(ac == 1),
                                     tile_position=(0, 64))
                ob = hsb.tile([128, 512], f16, name="ob", tag="ob", bufs=4)
                nc.vector.tensor_tensor(ob[:], o_ps[:],
                                        rso_sb[b * 8 + pr][:], MULT)
                if pr % 2 == 0:
                    nc.scalar.dma_start(OUT[b, pr], ob[:])
                else:
                    nc.sync.dma_start(OUT[b, pr], ob[:])

    nc.compile()
    return nc


def _prep_inputs(x, w_qkv, agent, w_noise, b_noise, w_mask, b_mask,
                 w_thresh, b_thresh):
    scale = D ** -0.5
    xt = np.ascontiguousarray(
        x.transpose(0, 2, 1).astype(np.float16)).reshape(B * 4, 128, N)
    wq = w_qkv[0:H * D].reshape(H, D, DIM)
    wk = w_qkv[H * D:2 * H * D].reshape(H, D, DIM)
    wv = w_qkv[2 * H * D:3 * H * D].reshape(H, D, DIM)
    bn_rep = np.ascontiguousarray(
        np.broadcast_to(b_noise[None, :], (128, 64))).astype(np.float32)
    bm_rep = np.ascontiguousarray(
        np.broadcast_to(b_mask[None, :], (128, 64))).astype(np.float32)
    wtt = np.zeros((A, D), np.float32)
    for a in range(A):
        wtt[a] = w_thresh[0, (a % 8) * D:(a % 8 + 1) * D]
    wtt = wtt.reshape(2, 128, 64)
    ident = np.eye(128, dtype=np.float32)
    vones = np.zeros((16, N), np.float16)
    vones[0, :] = 1.0
    bthr = np.full((128, 1), float(np.asarray(b_thresh).ravel()[0]), np.float32)
    in_maps = []
    for h in range(H):
        wqk_h = np.concatenate([wq[h], wk[h]], axis=0)            # [128, 512]
        wqk_t = np.ascontiguousarray(wqk_h.T).astype(np.float16)  # [512, 128]
        wv_t = np.ascontiguousarray(wv[h].T).astype(np.float16)   # [512, 64]
        ags = np.ascontiguousarray(np.concatenate(
            [agent[h].T * scale, agent[h].T], axis=0)).astype(np.float16)
        in_maps.append({
            "xt": xt,
            "wqk": np.ascontiguousarray(wqk_t.reshape(4, 128, 128)),
            "wv": np.ascontiguousarray(wv_t.reshape(4, 128, 64)),
            "ags": ags,
            "wn": np.ascontiguousarray(w_noise.T).astype(np.float32),
            "wm": np.ascontiguousarray(w_mask.T).astype(np.float32),
            "bn": bn_rep,
            "bm": bm_rep,
            "wtt": wtt,
            "ident": ident,
            "bthr": bthr,
            "vones": vones,
        })
    return in_maps


LAST_EXEC_NS = None
LAST_RES = None


def kernel(**inputs):
    global LAST_EXEC_NS, LAST_RES
    _install_profile_shim()
    if "nc" not in _cache:
        _cache["nc"] = _build()
    nc = _cache["nc"]
    inputs = {k: np.asarray(v) for k, v in inputs.items()}
    in_maps = _prep_inputs(**inputs)
    trace = os.environ.get("BASS_KERNEL_TRACE", "0") == "1"
    res = bass_utils.run_bass_kernel_spmd(
        nc, in_maps, core_ids=list(range(N_CORES)), trace=trace)
    LAST_EXEC_NS = res.exec_time_ns
    LAST_RES = res
    out = np.empty((B, N, H * D), np.float32)
    for h in range(H):
        o = res.results[h]["out_t"]          # [B, 8, 128, 512]
        # row p<64 of pair pr -> (n = 1024*pr + c, d = p);
        # row p>=64        -> (n = 1024*pr + 512 + c, d = p - 64)
        o2 = o.reshape(B, 8, 2, 64, 512)     # [b, pr, half, d, c]
        o3 = o2.transpose(0, 1, 2, 4, 3).reshape(B, N, D)
        out[:, :, h * D:(h + 1) * D] = o3
    return out


# revision 42
# speedup vs baseline: 1.2978x; 1.2978x over previous
"""Trainium2 Bass kernel for nn_Agent_Aggregator_with_Mask_Denoise_Mechanism.

Sharding: tensor-parallel over heads h (8 heads -> 8 cores). Each core computes
its head end-to-end; the only cross-core value is the scalar threshold logit
(an 8-way AllReduce of a 4-byte partial sum, padded to 64 B). Host does the
final (h d) concat + transpose.

Math notes vs the reference:
- sigmoid(m) > sigmoid(t)  <=>  m > t, so the mask threshold compares
  pre-sigmoid logits and no sigmoid tables are needed (Exp only).
- The reference's kv.reshape(b, a, h*d) row-major remap means the thresh
  weight applied to kv[b,h,a,d] is w_thresh[0, (a%8)*64+d], independent of h.
- Softmax normalizations are folded into matmuls (ones columns / ones blocks);
  each division happens on the small side of its matmul.
- ka softmax skips max-subtraction (fp32 psum); a fixed shift of -14 keeps
  exp outputs inside fp16 range for the kv matmul.

Schedule notes:
- The AllReduce costs ~40us wall regardless of payload, so it is launched
  right after the kv epilogue (phase D) and overlapped with the qa phase
  (F: logits+exp) plus the qa-softmax denominators (H-s) and the
  threshold-independent parts of the mask/denoise phase (G-pre).
- v projections for two adjacent 512-col chunks run as a PE column-group
  pair (tile_position (0,0)/(0,64)) so both halves of the PE array stream
  concurrently; same pattern as the paired output matmuls in phase H.
"""
import os
import sys

sys.path.insert(0, "/opt/trn_rl_repo")

import numpy as np
from contextlib import ExitStack

import concourse.bass as bass
import concourse.tile as tile
from concourse import bacc, mybir, bass_utils

f32 = mybir.dt.float32
f16 = mybir.dt.float16

B, N, DIM = 2, 8192, 512
H, A, D = 8, 256, 64
N_CORES = 8
NBLK = 4            # 2048-column blocks per batch
SHIFT = -14.0       # ka exp shift to fit fp16

_cache = {}


def _install_profile_shim():
    """Restore the axon NTFF profile hook + disable artifact upload."""
    import contextlib
    import ctypes
    import types

    if "antenv.axon_hooks" in sys.modules:
        return
    so_path = "/opt/axon/libaxon_pjrt.so"
    holder = [None]
    mod = types.ModuleType("antenv.axon_hooks")
    mod.set_axon_ntff_profile_hook = lambda h: holder.__setitem__(0, h)
    mod.get_axon_ntff_profile_hook = lambda: holder[0]
    sys.modules["antenv.axon_hooks"] = mod
    try:
        lib = ctypes.CDLL(so_path)
        if hasattr(lib, "axon_start_nrt_profile"):
            lib.axon_start_nrt_profile.argtypes = [
                ctypes.POINTER(ctypes.c_int64),
                ctypes.c_size_t,
            ]
            lib.axon_start_nrt_profile.restype = ctypes.c_int64
            lib.axon_stop_nrt_profile.argtypes = [ctypes.c_char_p]
            lib.axon_stop_nrt_profile.restype = ctypes.c_int64

            @contextlib.contextmanager
            def _hook(output_dir, device_ids):
                import jax

                jax.devices()
                if device_ids:
                    ids = (ctypes.c_int64 * len(device_ids))(*device_ids)
                    rc = lib.axon_start_nrt_profile(ids, len(device_ids))
                else:
                    rc = lib.axon_start_nrt_profile(None, 0)
                if rc != 0:
                    raise RuntimeError(f"axon_start_nrt_profile rc={rc}")
                try:
                    yield
                finally:
                    n = lib.axon_stop_nrt_profile(str(output_dir).encode())
                    if n < 0:
                        raise RuntimeError(f"axon_stop_nrt_profile rc={n}")

            mod.set_axon_ntff_profile_hook(_hook)
    except OSError:
        pass
    bass_utils.upload_artifacts = lambda tmpdir: f"file://{tmpdir}"


def _build():
    nc = bacc.Bacc("TRN2", target_bir_lowering=False, debug=False,
                   num_devices=N_CORES)

    XT = nc.dram_tensor("xt", [B * 4, 128, N], f16, kind="ExternalInput").ap()
    WQK = nc.dram_tensor("wqk", [4, 128, 128], f16, kind="ExternalInput").ap()
    WV = nc.dram_tensor("wv", [4, 128, 64], f16, kind="ExternalInput").ap()
    AGS = nc.dram_tensor("ags", [128, 256], f16, kind="ExternalInput").ap()
    WN = nc.dram_tensor("wn", [64, 64], f32, kind="ExternalInput").ap()
    WM = nc.dram_tensor("wm", [64, 64], f32, kind="ExternalInput").ap()
    BN = nc.dram_tensor("bn", [128, 64], f32, kind="ExternalInput").ap()
    BM = nc.dram_tensor("bm", [128, 64], f32, kind="ExternalInput").ap()
    WTT = nc.dram_tensor("wtt", [2, 128, 64], f32, kind="ExternalInput").ap()
    IDENT = nc.dram_tensor("ident", [128, 128], f32, kind="ExternalInput").ap()
    BTHR = nc.dram_tensor("bthr", [128, 1], f32, kind="ExternalInput").ap()
    VONES = nc.dram_tensor("vones", [16, N], f16, kind="ExternalInput").ap()
    OUT = nc.dram_tensor("out_t", [B, 8, 128, 512], f16,
                         kind="ExternalOutput").ap()

    EXP = mybir.ActivationFunctionType.Exp
    MULT = mybir.AluOpType.mult
    ADD = mybir.AluOpType.add
    GT = mybir.AluOpType.is_gt

    with tile.TileContext(nc) as tc, ExitStack() as ctx:
        const = ctx.enter_context(tc.tile_pool(name="const", bufs=1))
        big = ctx.enter_context(tc.tile_pool(name="big", bufs=1))
        ek = ctx.enter_context(tc.tile_pool(name="ek", bufs=3))
        dsb = ctx.enter_context(tc.tile_pool(name="dsb", bufs=1))
        hsb = ctx.enter_context(tc.tile_pool(name="hsb", bufs=2))
        dram = ctx.enter_context(tc.tile_pool(name="dram", bufs=1, space="DRAM"))
        ps_qk = ctx.enter_context(tc.tile_pool(name="ps_qk", bufs=2, space="PSUM"))
        ps_v = ctx.enter_context(tc.tile_pool(name="ps_v", bufs=2, space="PSUM"))
        ps_lg = ctx.enter_context(tc.tile_pool(name="ps_lg", bufs=2, space="PSUM"))
        ps_kvt = ctx.enter_context(tc.tile_pool(name="ps_kvt", bufs=2, space="PSUM"))

        # ---- constants to SBUF
        wqk_sb = []
        wv_sb = []
        for dc in range(4):
            w1 = const.tile([128, 128], f16, name=f"wqk{dc}")
            nc.scalar.dma_start(w1[:], WQK[dc])
            wqk_sb.append(w1)
            w2 = const.tile([128, 64], f16, name=f"wv{dc}")
            nc.scalar.dma_start(w2[:], WV[dc])
            wv_sb.append(w2)
        ags_sb = const.tile([128, 256], f16)
        nc.scalar.dma_start(ags_sb[:], AGS[:])
        wn_sb = const.tile([64, 64], f32)
        nc.scalar.dma_start(wn_sb[:], WN[:])
        wm_sb = const.tile([64, 64], f32)
        nc.scalar.dma_start(wm_sb[:], WM[:])
        bn_sb = const.tile([128, 64], f32)
        nc.scalar.dma_start(bn_sb[:], BN[:])
        bm_sb = const.tile([128, 64], f32)
        nc.scalar.dma_start(bm_sb[:], BM[:])
        wtt_sb = []
        for ac in range(2):
            w3 = const.tile([128, 64], f32, name=f"wtt{ac}")
            nc.scalar.dma_start(w3[:], WTT[ac])
            wtt_sb.append(w3)
        id_sb = const.tile([128, 128], f32)
        nc.scalar.dma_start(id_sb[:], IDENT[:])
        bthr_sb = const.tile([128, 1], f32)
        nc.scalar.dma_start(bthr_sb[:], BTHR[:])
        bias_sh = const.tile([128, 1], f32)
        nc.vector.memset(bias_sh[:], SHIFT)
        ones64 = const.tile([128, 64], f16)
        nc.vector.memset(ones64[:], 1.0)
        ones128 = nc.const_aps.tensor(1.0, [128, 1])


        # ---- persistent big tiles
        qkT = [big.tile([128, N], f16, name=f"qkT{b}") for b in range(B)]
        vsb = [big.tile([128, 64 * 80], f16, name=f"vsb{b}") for b in range(B)]
        vsb3 = [t[:].rearrange("p (c e) -> p c e", e=80) for t in vsb]
        eqa = [big.tile([128, N], f16, name=f"eqa{b}{ac}")
               for b in range(B) for ac in range(2)]
        rso_sb = [big.tile([128, 512], f16, name=f"rso{b}{pr}")
                  for b in range(B) for pr in range(8)]

        # ---- phase-D state (filled by emit_d)
        noise_sb = {}
        mask_sb = {}
        kv_sb = {}
        rs_sb = {}
        r_sb = {}
        zm_sb = {}
        den_sb = {}

        def emit_d(b, kvt_ps):
            t_u = dsb.tile([65, 256], f32, name=f"kvut{b}")
            nc.vector.tensor_copy(t_u[:], kvt_ps[b][:])
            for ac in range(2):
                asl = slice(ac * 128, (ac + 1) * 128)
                sm1 = ps_v.tile([128, 65], f32, name="sm1", tag="vps")
                nc.tensor.matmul(sm1[:, 0:64], t_u[0:64, asl], wn_sb[:],
                                 start=True, stop=True)
                t_n = dsb.tile([128, 64], f32, name=f"noise{b}{ac}")
                nc.vector.tensor_copy(t_n[:], sm1[:, 0:64])
                noise_sb[b, ac] = t_n
                sm2 = ps_v.tile([128, 65], f32, name="sm2", tag="vps")
                nc.tensor.matmul(sm2[:, 0:64], t_u[0:64, asl], wm_sb[:],
                                 start=True, stop=True)
                t_m = dsb.tile([128, 64], f32, name=f"mask{b}{ac}")
                nc.vector.tensor_copy(t_m[:], sm2[:, 0:64])
                mask_sb[b, ac] = t_m
                sm3 = ps_v.tile([128, 65], f32, name="sm3", tag="vps")
                nc.tensor.transpose(sm3[:], t_u[:, asl], id_sb[0:65, 0:65])
                t_k = dsb.tile([128, 65], f32, name=f"kvn{b}{ac}")
                nc.vector.tensor_copy(t_k[:], sm3[:])
                t_rs = dsb.tile([128, 1], f32, name=f"rs{b}{ac}")
                nc.vector.reciprocal_approx_fast(t_rs[:], t_k[:, 64:65])
                rs_sb[b, ac] = t_rs
                t_kv = dsb.tile([128, 64], f32, name=f"kv{b}{ac}")
                nc.vector.tensor_scalar(out=t_kv[:], in0=t_k[:, 0:64],
                                        scalar1=t_rs[:], scalar2=None, op0=MULT)
                kv_sb[b, ac] = t_kv
                t_tmp = dsb.tile([128, 64], f32, name=f"tt{b}{ac}")
                nc.vector.tensor_tensor(t_tmp[:], t_kv[:], wtt_sb[ac][:], MULT)
                t_r = dsb.tile([128, 1], f32, name=f"r{b}{ac}")
                nc.vector.tensor_reduce(t_r[:], t_tmp[:],
                                        axis=mybir.AxisListType.X, op=ADD)
                r_sb[b, ac] = t_r

        def emit_g_pre(b, ac):
            # threshold-independent parts of the mask/denoise epilogue
            t_rs = rs_sb[b, ac]
            zm = dsb.tile([128, 64], f32, name=f"zm{b}{ac}")
            nc.vector.scalar_tensor_tensor(
                out=zm[:], in0=mask_sb[b, ac][:], scalar=t_rs[:],
                in1=bm_sb[:], op0=MULT, op1=ADD)
            zm_sb[b, ac] = zm
            gn = dsb.tile([128, 64], f32, name=f"gn{b}{ac}")
            nc.vector.scalar_tensor_tensor(
                out=gn[:], in0=noise_sb[b, ac][:], scalar=t_rs[:],
                in1=bn_sb[:], op0=MULT, op1=ADD)
            en = dsb.tile([128, 64], f32, name=f"en{b}{ac}")
            nc.scalar.activation(en[:], gn[:], EXP, scale=-1.0)
            dd = dsb.tile([128, 64], f32, name=f"dd{b}{ac}")
            nc.vector.tensor_scalar(out=dd[:], in0=en[:], scalar1=1.0,
                                    scalar2=None, op0=ADD)
            den = dsb.tile([128, 64], f32, name=f"den{b}{ac}")
            nc.vector.reciprocal_approx_fast(den[:], dd[:])
            den_sb[b, ac] = den

        def emit_f(b, pr):
            # qa logits + exp for one 1024-token pair, then the qa-softmax
            # denominator matmuls (H-s) and their reciprocal.
            sl0 = slice((2 * pr) * 512, (2 * pr + 1) * 512)
            sl1 = slice((2 * pr + 1) * 512, (2 * pr + 2) * 512)
            for sl in (sl0, sl1):
                for ac in range(2):
                    lgq = ps_lg.tile([128, 512], f32, name="lgq", tag="lg")
                    nc.tensor.matmul(
                        lgq[:], ags_sb[0:64, ac * 128:(ac + 1) * 128],
                        qkT[b][0:64, sl], start=True, stop=True)
                    nc.scalar.activation(eqa[b * 2 + ac][:, sl], lgq[:], EXP)
            s_ps = ps_qk.tile([128, 512], f32, name="sps", tag="qkps")
            for ac in range(2):
                eq = eqa[b * 2 + ac]
                nc.tensor.matmul(s_ps[0:64, :], ones64[:], eq[:, sl0],
                                 start=(ac == 0), stop=(ac == 1),
                                 tile_position=(0, 0))
                nc.tensor.matmul(s_ps[64:128, :], ones64[:], eq[:, sl1],
                                 start=(ac == 0), stop=(ac == 1),
                                 tile_position=(0, 64))
            rtmp = hsb.tile([128, 512], f32, name="rtmp", tag="rtmp", bufs=1)
            nc.vector.reciprocal_approx_fast(rtmp[:], s_ps[:])
            nc.vector.tensor_copy(rso_sb[b * 8 + pr][:], rtmp[:])

        # ====== main loop: A (proj) + B (v transpose) + C (ka) ======
        N_F_EARLY = 3   # F groups for b=0 emitted inside the last b=1 block
        with ExitStack() as sA:
            xtp = sA.enter_context(tc.tile_pool(name="xtp", bufs=2))
            kvt_ps = [ps_kvt.tile([65, 256], f32, name=f"kvtps{b}", tag="kvtps")
                      for b in range(B)]
            kv_mm_idx = [0, 0]
            # pre-issue all x loads on gpsimd (SWDGE) so they never queue
            # behind exp/transpose work; pool WAR deps pace the prefetch.
            # x prefetch: half-block [128, 1024] tiles, 4-deep per dc tag;
            # dc0/dc1 on the gpsimd queue, dc2/dc3 on sync so the issue
            # rate keeps up with consumption.  (An early warmup AllReduce
            # was tried to hide the CC cold-start, but collectives occupy
            # the Q7 cores that serve SWDGE, which stalls the vt transposes
            # for the whole warmup -- a net loss.)
            xts_all = {}
            pairs = [(blk, b) for blk in range(NBLK) for b in range(B)]
            for blk in range(NBLK):
                for b in range(B):
                    for hf in range(2):
                        bsl = slice(blk * 2048 + hf * 1024,
                                    blk * 2048 + (hf + 1) * 1024)
                        for dc in range(4):
                            xt_t = xtp.tile([128, 1024], f16,
                                            name=f"x{blk}{b}{dc}{hf}",
                                            tag=f"x{dc}", bufs=4)
                            eng = nc.gpsimd if dc < 2 else nc.sync
                            eng.dma_start(xt_t[:], XT[b * 4 + dc][:, bsl])
                            xts_all[blk, b, dc, hf] = xt_t

            # persistent ping-pong vt tiles; their VONES rows are loaded once
            vts = [big.tile([80, 2048], f16, name=f"vt{i}") for i in range(2)]
            for i in range(2):
                nc.sync.dma_start(vts[i][64:80, :], VONES[:, 0:2048])

            def emit_v(it):
                # v projection + transposes for iteration `it`, software-
                # pipelined one iteration ahead of its consuming C phase.
                blk, b = pairs[it]
                vt = vts[it % 2]
                for pr2 in range(2):
                    sse = slice((2 * pr2) * 512, (2 * pr2 + 1) * 512)
                    sso = slice((2 * pr2 + 1) * 512, (2 * pr2 + 2) * 512)
                    xh = [xts_all[blk, b, dc, pr2] for dc in range(4)]
                    v2_ps = ps_v.tile([128, 512], f32, name="vps", tag="vps")
                    for dc in range(4):
                        nc.tensor.matmul(v2_ps[0:64, :], wv_sb[dc][:],
                                         xh[dc][:, 0:512],
                                         start=(dc == 0), stop=(dc == 3),
                                         tile_position=(0, 0))
                        nc.tensor.matmul(v2_ps[64:128, :], wv_sb[dc][:],
                                         xh[dc][:, 512:1024],
                                         start=(dc == 0), stop=(dc == 3),
                                         tile_position=(0, 64))
                    nc.vector.tensor_copy(vt[0:64, sse], v2_ps[0:64, :])
                    nc.vector.tensor_copy(vt[0:64, sso], v2_ps[64:128, :])
                    hsl = slice(pr2 * 1024, (pr2 + 1) * 1024)
                    nc.sync.dma_start_transpose(
                        vsb3[b][:, blk * 16 + pr2 * 8:
                                 blk * 16 + (pr2 + 1) * 8, :], vt[:, hsl])

            for it, (blk, b) in enumerate(pairs):
                emit_v(it)
                for sc in range(4):
                    nck = blk * 4 + sc
                    sl = slice(nck * 512, (nck + 1) * 512)
                    qk_ps = ps_qk.tile([128, 512], f32, name="qkps",
                                       tag="qkps")
                    for dc in range(4):
                        nc.tensor.matmul(
                            qk_ps[:], wqk_sb[dc][:],
                            xts_all[blk, b, dc, sc // 2][
                                :, (sc % 2) * 512:(sc % 2 + 1) * 512],
                            start=(dc == 0), stop=(dc == 3))
                    nc.vector.tensor_copy(qkT[b][:, sl], qk_ps[:])
                # early F groups for b=0 fill the end-of-loop drain while
                # the last block's transpose + C chain completes
                if it == len(pairs) - 1:
                    for pr in range(N_F_EARLY):
                        emit_f(0, pr)
                # C: ka logits -> exp -> kv^T
                for cp in range(blk * 8, (blk + 1) * 8):
                    lg = ps_lg.tile([128, 512], f32, name="lg", tag="lg")
                    for j in range(2):
                        c = 2 * cp + j
                        nc.tensor.matmul(
                            lg[:, j * 256:(j + 1) * 256],
                            qkT[b][64:128, c * 128:(c + 1) * 128],
                            ags_sb[64:128, :],
                            start=True, stop=True)
                    e_t = ek.tile([128, 512], f16, name="eka", tag="eka")
                    nc.scalar.activation(e_t[:], lg[:], EXP, bias=bias_sh[:])
                    for j in range(2):
                        c = 2 * cp + j
                        ki = kv_mm_idx[b]
                        nc.tensor.matmul(
                            kvt_ps[b][:], vsb3[b][:, c, 0:65],
                            e_t[:, j * 256:(j + 1) * 256],
                            start=(ki == 0), stop=(ki == 63))
                        kv_mm_idx[b] += 1
                # D: per-batch epilogue right after its last C block
                if blk == NBLK - 1:
                    emit_d(b, kvt_ps)

            # ---- threshold partial + cross-core AllReduce (launch ASAP;
            # phases F / H-s / G-pre below overlap its ~40us latency)
            th_ps = ps_v.tile([1, 16], f32, name="thps", tag="vps")
            k = 0
            for b in range(B):
                for ac in range(2):
                    nc.tensor.matmul(th_ps[0:1, 0:1], r_sb[b, ac][:],
                                     ones128[0:128, :],
                                     start=(k == 0), stop=(k == 3))
                    k += 1
            th_sb = dsb.tile([1, 16], f32)
            nc.vector.memset(th_sb[:], 0.0)
            nc.vector.tensor_copy(th_sb[0:1, 0:1], th_ps[0:1, 0:1])
            cc_in = dram.tile([1, 16], f32)
            cc_out = dram.tile([1, 16], f32, addr_space="Shared")
            nc.sync.dma_start(cc_in[:], th_sb[:])
            nc.gpsimd.collective_compute(
                "AllReduce", ADD, ins=[cc_in[:]], outs=[cc_out[:]],
                replica_groups=[list(range(N_CORES))])

            # ---- G-pre: threshold-independent mask/denoise pieces
            for b in range(B):
                for ac in range(2):
                    emit_g_pre(b, ac)

            # ---- F: qa logits -> exp, plus qa-softmax denominators (H-s),
            # all overlapped with the AllReduce.
            for b in range(B):
                for pr in range(N_F_EARLY if b == 0 else 0, 8):
                    emit_f(b, pr)

            # ---- collective result -> threshold scalar
            ts_sb = dsb.tile([1, 16], f32)
            nc.sync.dma_start(ts_sb[:], cc_out[:])
            tbc = dsb.tile([128, 1], f32)
            nc.gpsimd.partition_broadcast(tbc[:], ts_sb[0:1, 0:1])
            tfin = dsb.tile([128, 1], f32)
            nc.vector.tensor_scalar(out=tfin[:], in0=tbc[:],
                                    scalar1=1.0 / (B * A), scalar2=bthr_sb[:],
                                    op0=MULT, op1=ADD)

        # ========== phase G: mask -> kv2 (only thresh-dependent part),
        # interleaved per batch with phase H so H(b=0) starts while G(b=1)
        # still runs on the vector/scalar engines.
        kv2_sb = {}

        def emit_g(b):
            for ac in range(2):
                t_kv = kv_sb[b, ac]
                mb = dsb.tile([128, 64], f32, name=f"mb{b}{ac}")
                nc.vector.tensor_scalar(out=mb[:], in0=zm_sb[b, ac][:],
                                        scalar1=tfin[:],
                                        scalar2=None, op0=GT)
                kvm = dsb.tile([128, 64], f32, name=f"kvm{b}{ac}")
                nc.vector.tensor_tensor(kvm[:], t_kv[:], mb[:], MULT)
                l2 = dsb.tile([128, 64], f32, name=f"l2{b}{ac}")
                nc.vector.tensor_tensor(l2[:], kvm[:], den_sb[b, ac][:], ADD)
                e2 = dsb.tile([128, 64], f32, name=f"e2{b}{ac}")
                s2 = dsb.tile([128, 1], f32, name=f"s2{b}{ac}")
                nc.scalar.activation(e2[:], l2[:], EXP, accum_out=s2[:])
                rs2 = dsb.tile([128, 1], f32, name=f"rs2{b}{ac}")
                nc.vector.reciprocal_approx_fast(rs2[:], s2[:])
                kv2 = dsb.tile([128, 64], f16, name=f"kv2_{b}{ac}")
                nc.vector.tensor_scalar(out=kv2[:], in0=e2[:],
                                        scalar1=rs2[:], scalar2=None, op0=MULT)
                kv2_sb[b * 2 + ac] = kv2

        # ===== phase H: out^T = kv2^T @ E_qa^T, paired via column tiling ====
        # Chunk pairs land on disjoint PSUM partition halves.  Denominators
        # were already folded into rso_sb during phase F.
        emit_g(0)
        for b in range(B):
            if b + 1 < B:
                emit_g(b + 1)
            for pr in range(8):
                sl0 = slice((2 * pr) * 512, (2 * pr + 1) * 512)
                sl1 = slice((2 * pr + 1) * 512, (2 * pr + 2) * 512)
                o_ps = ps_lg.tile([128, 512], f32, name="ops", tag="lg")
                for ac in range(2):
                    k2 = kv2_sb[b * 2 + ac]
                    eq = eqa[b * 2 + ac]
                    nc.tensor.matmul(o_ps[0:64, :], k2[:], eq[:, sl0],
                                     start=(ac == 0), stop=(ac == 1),
                                     tile_position=(0, 0))
                    nc.tensor.matmul(o_ps[64:128, :], k2[:], eq[:, sl1],
                                     start=(ac == 0), stop=(ac == 1),
                                     tile_position=(0, 64))
                ob = hsb.tile([128, 512], f16, name="ob", tag="ob", bufs=4)
                nc.vector.tensor_tensor(ob[:], o_ps[:],
                                        rso_sb[b * 8 + pr][:], MULT)
                if pr % 2 == 0:
                    nc.scalar.dma_start(OUT[b, pr], ob[:])
                else:
                    nc.sync.dma_start(OUT[b, pr], ob[:])

    nc.compile()
    return nc


def _prep_inputs(x, w_qkv, agent, w_noise, b_noise, w_mask, b_mask,
                 w_thresh, b_thresh):
    scale = D ** -0.5
    xt = np.ascontiguousarray(
        x.transpose(0, 2, 1).astype(np.float16)).reshape(B * 4, 128, N)
    wq = w_qkv[0:H * D].reshape(H, D, DIM)
    wk = w_qkv[H * D:2 * H * D].reshape(H, D, DIM)
    wv = w_qkv[2 * H * D:3 * H * D].reshape(H, D, DIM)
    bn_rep = np.ascontiguousarray(
        np.broadcast_to(b_noise[None, :], (128, 64))).astype(np.float32)
    bm_rep = np.ascontiguousarray(
        np.broadcast_to(b_mask[None, :], (128, 64))).astype(np.float32)
    wtt = np.zeros((A, D), np.float32)
    for a in range(A):
        wtt[a] = w_thresh[0, (a % 8) * D:(a % 8 + 1) * D]
    wtt = wtt.reshape(2, 128, 64)
    ident = np.eye(128, dtype=np.float32)
    vones = np.zeros((16, N), np.float16)
    vones[0, :] = 1.0
    bthr = np.full((128, 1), float(np.asarray(b_thresh).ravel()[0]), np.float32)
    in_maps = []
    for h in range(H):
        wqk_h = np.concatenate([wq[h], wk[h]], axis=0)            # [128, 512]
        wqk_t = np.ascontiguousarray(wqk_h.T).astype(np.float16)  # [512, 128]
        wv_t = np.ascontiguousarray(wv[h].T).astype(np.float16)   # [512, 64]
        ags = np.ascontiguousarray(np.concatenate(
            [agent[h].T * scale, agent[h].T], axis=0)).astype(np.float16)
        in_maps.append({
            "xt": xt,
            "wqk": np.ascontiguousarray(wqk_t.reshape(4, 128, 128)),
            "wv": np.ascontiguousarray(wv_t.reshape(4, 128, 64)),
            "ags": ags,
            "wn": np.ascontiguousarray(w_noise.T).astype(np.float32),
            "wm": np.ascontiguousarray(w_mask.T).astype(np.float32),
            "bn": bn_rep,
            "bm": bm_rep,
            "wtt": wtt,
            "ident": ident,
            "bthr": bthr,
            "vones": vones,
        })
    return in_maps


LAST_EXEC_NS = None
LAST_RES = None


def kernel(**inputs):
    global LAST_EXEC_NS, LAST_RES
    _install_profile_shim()
    if "nc" not in _cache:
        _cache["nc"] = _build()
    nc = _cache["nc"]
    inputs = {k: np.asarray(v) for k, v in inputs.items()}
    in_maps = _prep_inputs(**inputs)
    trace = os.environ.get("BASS_KERNEL_TRACE", "0") == "1"
    res = bass_utils.run_bass_kernel_spmd(
        nc, in_maps, core_ids=list(range(N_CORES)), trace=trace)
    LAST_EXEC_NS = res.exec_time_ns
    LAST_RES = res
    out = np.empty((B, N, H * D), np.float32)
    for h in range(H):
        o = res.results[h]["out_t"]          # [B, 8, 128, 512]
        # row p<64 of pair pr -> (n = 1024*pr + c, d = p);
        # row p>=64        -> (n = 1024*pr + 512 + c, d = p - 64)
        o2 = o.reshape(B, 8, 2, 64, 512)     # [b, pr, half, d, c]
        o3 = o2.transpose(0, 1, 2, 4, 3).reshape(B, N, D)
        out[:, :, h * D:(h + 1) * D] = o3
    return out
